# revision 1
# baseline (speedup 1.0000x reference)
"""3-layer GCN (GCNConv x3, PyG-default norm) on 8 Trainium2 NeuronCores.

Dst-sharded design. Nodes are packed into 128-row blocks via vector
bin-packing on per-quadrant in-edge counts; each layer's "table" holds
dinv[n]*(h[n] @ W) rows for all nodes (101376 rows incl. zero blocks).
Slots (in-edges + self-loop) are gathered per (block-group, quadrant) with
dma_gather (int16 quadrant-local indices) and summed with strided
tensor_reduce; epilogue applies dinv, bias, relu on DVE/ACT, and the
PE transposes + matmuls produce the next layer's table shard.
The inter-layer shard exchange is host-mediated (concat of per-core
outputs between device runs).
"""

import numpy as np

N = 100000
D = 64
NCORES = 8
NPC = N // NCORES
P = 128
NB = 98
SH = (NB + 1) * P            # 12672 rows/shard (incl zero block)
TBL = NCORES * SH            # 101376
QROWS = 2 * SH               # 25344 int16-addressable rows
PADLOC = NB * P              # zero-block row (shard-low of quadrant)
COLS_CAP = 96
GMAX = 6

_CACHE = {}


def _plan(edge_index):
    key = hash(edge_index.tobytes())
    if key in _CACHE:
        return _CACHE[key]
    src = np.asarray(edge_index[0], dtype=np.int64)
    dst = np.asarray(edge_index[1], dtype=np.int64)
    deg = (np.bincount(dst, minlength=N) + 1).astype(np.float64)
    dinv = (1.0 / np.sqrt(deg)).astype(np.float32)

    qcnt_all = np.zeros((N, 4), dtype=np.int64)
    np.add.at(qcnt_all, (dst, src // (2 * NPC)), 1)
    qcnt_all[np.arange(N), np.arange(N) // (2 * NPC)] += 1   # self slot

    def _pack(nodes, qc, budget):
        order = np.argsort(-(qc.max(1) * 100 + qc.sum(1)), kind="stable")
        bmax = np.zeros((NB, 4), dtype=np.int64)
        bfill = np.zeros(NB, dtype=np.int64)
        members = [[] for _ in range(NB)]
        for i in order:
            v = qc[i]
            eff = np.maximum(bmax, budget)      # free headroom up to budget
            inc = (np.maximum(eff, v[None, :]) - eff).sum(1)
            inc[bfill >= P] = 1 << 40
            j = int(np.argmin(inc + bfill * 1e-4))
            members[j].append(nodes[i])
            bmax[j] = np.maximum(bmax[j], v)
            bfill[j] += 1
        return members, bmax

    # pass 1: independent packing -> median per-block budget across cores
    prof = np.zeros((NCORES, NB, 4), dtype=np.int64)
    for c in range(NCORES):
        nodes = np.arange(c * NPC, (c + 1) * NPC)
        _, bmax = _pack(nodes, qcnt_all[nodes], np.zeros((NB, 4), np.int64))
        bo = np.argsort(-bmax.sum(1), kind="stable")
        prof[c] = bmax[bo]
    budget = np.median(prof, axis=0).astype(np.int64)

    # pass 2: re-pack every core against the shared budgets
    perms = []                     # node id per position [12544], -1 = dummy
    pos_of = np.full(N, -1, dtype=np.int64)
    for c in range(NCORES):
        nodes = np.arange(c * NPC, (c + 1) * NPC)
        members, bmax = _pack(nodes, qcnt_all[nodes], budget)
        pe = np.full(NB * P, -1, dtype=np.int64)
        for b in range(NB):
            mem = members[b]
            pe[b * P:b * P + len(mem)] = mem
        valid = pe >= 0
        pos_of[pe[valid]] = np.flatnonzero(valid)
        perms.append(pe)

    per_core = []
    cnt_bq = np.zeros((NCORES, NB, 4), dtype=np.int64)
    for c in range(NCORES):
        m = (dst >= c * NPC) & (dst < (c + 1) * NPC)
        s_ = src[m]
        pp = pos_of[dst[m]]
        own = perms[c][perms[c] >= 0]
        ps = np.concatenate([pp, pos_of[own]])
        ss = np.concatenate([s_, own])
        qq = ss // (2 * NPC)
        loc = ((ss // NPC) % 2) * SH + pos_of[ss]
        o = np.lexsort((qq, ps))
        ps, qq, loc = ps[o], qq[o], loc[o]
        gkey = ps * 4 + qq
        starts = np.r_[0, np.flatnonzero(np.diff(gkey)) + 1]
        gid = np.cumsum(np.r_[True, np.diff(gkey) != 0]) - 1
        kk = np.arange(len(ps)) - starts[gid]
        cnt = np.bincount(gkey, minlength=NB * P * 4).reshape(NB * P, 4)
        cnt_bq[c] = cnt.reshape(NB, P, 4).max(1)
        per_core.append((ps, qq, kk, loc))

    Kbq = cnt_bq.max(0)

    groups = []
    b = 0
    while b < NB:
        G = 1
        K = Kbq[b].copy()
        while b + G < NB and G < GMAX:
            K2 = np.maximum(K, Kbq[b + G])
            if (G + 1) * int(K2.sum()) > COLS_CAP:
                break
            K = K2
            G += 1
        groups.append((b, G, K.astype(np.int64)))
        b += G

    idx16 = []
    calls = []
    c16 = 0
    for gi, (bs_, G, K) in enumerate(groups):
        qoff = 0
        for q in range(4):
            Kq = int(K[q])
            if Kq == 0:
                continue
            n = P * G * Kq
            calls.append((gi, q, c16, n, qoff))
            c16 += n // 16
            qoff += G * Kq
    TOTC16 = c16

    for c in range(NCORES):
        ps, qq, kk, loc = per_core[c]
        bb = ps // P
        pp = ps % P
        segs = []
        for (gi, q, c16s, n, qoff) in calls:
            bs_, G, K = groups[gi]
            Kq = int(K[q])
            V = np.full((P, G, Kq), PADLOC, dtype=np.int16)
            m = (bb >= bs_) & (bb < bs_ + G) & (qq == q)
            V[pp[m], bb[m] - bs_, kk[m]] = loc[m].astype(np.int16)
            flat = V.transpose(1, 2, 0).reshape(-1)
            segs.append(np.tile(flat.reshape(-1, 16).T, (NCORES, 1)))
        idx16.append(np.ascontiguousarray(np.concatenate(segs, axis=1)))
    plan = dict(dinv=dinv, perms=perms, groups=groups, calls=calls,
                idx16=idx16, TOTC16=TOTC16)
    _CACHE[key] = plan
    return plan


def _build_l0(plan):
    """Own-shard table build: tshard = blocks of xt_own @ W (xt pre-scaled)."""
    import concourse.bacc as bacc
    import concourse.mybir as mybir
    import concourse.tile as tile

    f32 = mybir.dt.float32
    nc = bacc.Bacc("TRN2", target_bir_lowering=False)
    xt_own = nc.dram_tensor("xt_own", [D, SH], f32, kind="ExternalInput")
    w_in = nc.dram_tensor("w0", [D, D], f32, kind="ExternalInput")
    tb = nc.dram_tensor("tshard", [SH, D], f32, kind="ExternalOutput")
    with tile.TileContext(nc) as tc:
        with (
            tc.tile_pool(name="sb", bufs=2) as sb,
            tc.tile_pool(name="cst", bufs=1) as cst,
            tc.tile_pool(name="ps", bufs=2, space="PSUM") as ps,
        ):
            ws = cst.tile([D, D], f32)
            nc.sync.dma_start(out=ws[:], in_=w_in[:])
            xt = cst.tile([D, SH], f32)
            nc.sync.dma_start(out=xt[:], in_=xt_own[:])
            for g0 in range(0, NB + 1, 8):
                gn = min(8, NB + 1 - g0)
                pst = ps.tile([P, 8 * D], f32, tag="ps")
                for j in range(gn):
                    blk = g0 + j
                    nc.tensor.matmul(
                        out=pst[:, j * D:(j + 1) * D],
                        lhsT=xt[:, blk * P:(blk + 1) * P],
                        rhs=ws[:], start=True, stop=True)
                stg = sb.tile([P, 8 * D], f32, tag="stg")
                nc.vector.tensor_copy(out=stg[:, :gn * D], in_=pst[:, :gn * D])
                nc.sync.dma_start(
                    out=tb[g0 * P:(g0 + gn) * P, :].rearrange("(g p) d -> p g d", p=P),
                    in_=stg[:, :gn * D])
    nc.compile()
    return nc


def _build_layer(plan, last):
    """One GCN layer: gather table -> reduce -> epilogue.
    Outputs h_out; if not last, also tshard = (dinv*h) @ Wn."""
    import concourse.bacc as bacc
    import concourse.mybir as mybir
    import concourse.tile as tile
    from concourse.masks import make_identity

    groups, calls, TOTC16 = plan["groups"], plan["calls"], plan["TOTC16"]
    f32 = mybir.dt.float32
    nc = bacc.Bacc("TRN2", target_bir_lowering=False, num_swdge_queues=2)
    table = nc.dram_tensor("table", [TBL, D], f32, kind="ExternalInput")
    idx_in = nc.dram_tensor("idx16", [P, TOTC16], mybir.dt.int16, kind="ExternalInput")
    dinv_in = nc.dram_tensor("dinvb", [P, NB], f32, kind="ExternalInput")
    bias_in = nc.dram_tensor("bias", [P, D], f32, kind="ExternalInput")
    wn_in = nc.dram_tensor("wn", [D, D], f32, kind="ExternalInput")
    h_out = nc.dram_tensor("h_out", [NB * P, D], f32, kind="ExternalOutput")
    if not last:
        tshard = nc.dram_tensor("tshard", [SH, D], f32, kind="ExternalOutput")

    with tile.TileContext(nc) as tc:
        with (
            tc.tile_pool(name="cst", bufs=1) as cst,
            tc.tile_pool(name="wk", bufs=3) as wk,
            tc.tile_pool(name="ep", bufs=2) as ep,
            tc.tile_pool(name="psT", bufs=2, space="PSUM") as psT,
            tc.tile_pool(name="psM", bufs=2, space="PSUM") as psM,
        ):
            idx16 = cst.tile([P, TOTC16], mybir.dt.int16)
            nc.sync.dma_start(out=idx16[:], in_=idx_in[:])
            dinvb = cst.tile([P, NB], f32)
            nc.sync.dma_start(out=dinvb[:], in_=dinv_in[:])
            bias_t = cst.tile([P, D], f32)
            nc.sync.dma_start(out=bias_t[:], in_=bias_in[:])
            wn = cst.tile([D, D], f32)
            nc.sync.dma_start(out=wn[:], in_=wn_in[:])
            ident = cst.tile([P, P], f32)
            make_identity(nc, ident[:])
            if not last:
                zb = cst.tile([P, D], f32)
                nc.vector.memset(zb[:], 0.0)
                nc.sync.dma_start(out=tshard[NB * P:(NB + 1) * P, :], in_=zb[:])

            for gi, (bstart, G, K) in enumerate(groups):
                COLS = G * int(K.sum())
                gbuf = wk.tile([P, COLS, D], f32, tag="gbuf")
                for (gi2, q, c16s, n, qoff) in calls:
                    if gi2 != gi:
                        continue
                    nc.gpsimd.dma_gather(
                        out_ap=gbuf[:, qoff:qoff + n // P, :],
                        in_ap=table[q * QROWS:(q + 1) * QROWS, :],
                        idxs_ap=idx16[:, c16s:c16s + n // 16],
                        num_idxs=n, num_idxs_reg=n, elem_size=D,
                        single_packet=False, queue_num=q % 2)
                acc = ep.tile([P, G, D], f32, tag="acc")
                tmp = ep.tile([P, G, D], f32, tag="tmp")
                first = True
                for (gi2, q, c16s, n, qoff) in calls:
                    if gi2 != gi:
                        continue
                    Kq = (n // P) // G
                    red_in = gbuf[:, qoff:qoff + G * Kq, :] \
                        .rearrange("p (g k) d -> p g d k", g=G)
                    nc.vector.tensor_reduce(
                        out=(acc if first else tmp)[:], in_=red_in,
                        axis=mybir.AxisListType.X, op=mybir.AluOpType.add)
                    if not first:
                        nc.vector.tensor_tensor(out=acc[:], in0=acc[:], in1=tmp[:],
                                                op=mybir.AluOpType.add)
                    first = False
                dvb = dinvb[:, bstart:bstart + G].to_broadcast([P, G, D])
                bias = bias_t[:].rearrange("p (g d) -> p g d", g=1) \
                    .to_broadcast([P, G, D])
                t1 = ep.tile([P, G, D], f32, tag="t1")
                nc.vector.tensor_tensor(out=t1[:], in0=acc[:], in1=dvb,
                                        op=mybir.AluOpType.mult)
                t2 = ep.tile([P, G, D], f32, tag="t2")
                nc.vector.tensor_tensor(out=t2[:], in0=t1[:], in1=bias,
                                        op=mybir.AluOpType.add)
                h = ep.tile([P, G, D], f32, tag="h")
                nc.scalar.activation(out=h[:], in_=t2[:],
                                     func=mybir.ActivationFunctionType.Relu)
                nc.sync.dma_start(
                    out=h_out[bstart * P:(bstart + G) * P, :]
                        .rearrange("(g p) d -> p g d", p=P),
                    in_=h[:])
                if not last:
                    hh = ep.tile([P, G, D], f32, tag="hh")
                    nc.vector.tensor_tensor(out=hh[:], in0=h[:], in1=dvb,
                                            op=mybir.AluOpType.mult)
                    agst = ep.tile([P, G, D], f32, tag="agst")
                    for b in range(G):
                        pt = psT.tile([D, P], f32, tag="pt")
                        nc.tensor.transpose(out=pt[:], in_=hh[:, b, :],
                                            identity=ident[:])
                        ht = ep.tile([D, P], f32, tag="ht")
                        nc.scalar.copy(out=ht[:], in_=pt[:])
                        pm = psM.tile([P, D], f32, tag="pm")
                        nc.tensor.matmul(out=pm[:], lhsT=ht[:], rhs=wn[:],
                                         start=True, stop=True)
                        nc.vector.tensor_copy(out=agst[:, b, :], in_=pm[:])
                    nc.sync.dma_start(
                        out=tshard[bstart * P:(bstart + G) * P, :]
                            .rearrange("(g p) d -> p g d", p=P),
                        in_=agst[:])
    nc.compile()
    return nc


def kernel(x, W1, b1, W2, b2, W3, b3, edge_index):
    import os
    from concourse.bass_utils import run_bass_kernel_spmd as _rb

    trace = os.environ.get("KERNEL_TRACE", "") == "1"
    times = []

    def run_bass_kernel_spmd(nc, in_maps, core_ids):
        import time as _t
        t0 = _t.time()
        try:
            r = _rb(nc, in_maps, core_ids=core_ids, trace=trace)
        except Exception:
            r = _rb(nc, in_maps, core_ids=core_ids)
        wall = (_t.time() - t0) * 1e9
        if r.exec_time_ns is not None:
            times.append(r.exec_time_ns)
            print(f"[kernel] run exec time: {r.exec_time_ns} ns")
        elif trace:
            times.append(int(wall))
            print(f"[kernel] run wall (incl dispatch): {wall/1e3:.0f} us")
        return r

    x = np.ascontiguousarray(np.asarray(x, dtype=np.float32))
    Ws = [np.ascontiguousarray(np.asarray(w, dtype=np.float32)) for w in (W1, W2, W3)]
    bs = [np.asarray(b, dtype=np.float32) for b in (b1, b2, b3)]
    plan = _plan(np.asarray(edge_index))
    dinv, perms = plan["dinv"], plan["perms"]
    cores = list(range(NCORES))

    if "nc0" not in plan:
        plan["nc0"] = _build_l0(plan)
        plan["ncl"] = {False: _build_layer(plan, False), True: _build_layer(plan, True)}
    nc0 = plan["nc0"]
    in0 = []
    for c in cores:
        pe = perms[c]
        valid = pe >= 0
        xt_own = np.zeros((D, SH), dtype=np.float32)
        xt_own[:, np.flatnonzero(valid)] = (x[pe[valid]] * dinv[pe[valid]][:, None]).T
        in0.append(dict(xt_own=xt_own, w0=Ws[0]))
    r0 = run_bass_kernel_spmd(nc0, in0, core_ids=cores)
    table = np.ascontiguousarray(
        np.concatenate([r0.results[c]["tshard"] for c in cores], axis=0))

    dinvbs = []
    for c in cores:
        pe = perms[c]
        valid = pe >= 0
        dv = np.zeros(NB * P, dtype=np.float32)
        dv[valid] = dinv[pe[valid]]
        dinvbs.append(np.ascontiguousarray(dv.reshape(NB, P).T))

    out = np.empty((N, D), dtype=np.float32)
    for lyr in range(3):
        last = lyr == 2
        ncl = plan["ncl"][last]
        wn = Ws[lyr + 1] if not last else Ws[0]
        bias = np.ascontiguousarray(np.tile(bs[lyr][None, :], (P, 1)))
        inl = [dict(table=table, idx16=plan["idx16"][c], dinvb=dinvbs[c],
                    bias=bias, wn=wn) for c in cores]
        rl = run_bass_kernel_spmd(ncl, inl, core_ids=cores)
        if not last:
            table = np.ascontiguousarray(
                np.concatenate([rl.results[c]["tshard"] for c in cores], axis=0))
        else:
            for c in cores:
                pe = perms[c]
                valid = pe >= 0
                out[pe[valid]] = rl.results[c]["h_out"][valid]
    if times:
        print(f"HW exec time: {sum(times)} ns")
    return out



# revision 2
# speedup vs baseline: 8.9822x; 8.9822x over previous
"""3-layer GCN (GCNConv x3, PyG-default norm) on 8 Trainium2 NeuronCores.

Single-launch dst-sharded design. Each core owns 12500 dst nodes laid out
degree-sorted into 98 blocks of 128 positions. Per layer, each core builds
its shard of the "table" (dinv[n]*(h[n] @ W) rows) with PE matmuls, the
shards are exchanged on-device with an 8-core AllGather collective, and
in-edge messages (+ self loop) are fetched per (block-group, quadrant)
with gpsimd dma_gather (int16 quadrant-local indices) and summed with
strided tensor_reduce. The epilogue applies dinv, bias, relu. All three
layers plus the exchanges run in ONE device launch; features ship as fp16
to cut host<->device transfer.
"""

import numpy as np

N = 100000
D = 64
NCORES = 8
NPC = N // NCORES
P = 128
NB = 98
SH = (NB + 1) * P            # 12672 rows/shard (incl zero block)
TBL = NCORES * SH            # 101376
QROWS = 2 * SH               # 25344 int16-addressable rows
PADLOC = NB * P              # zero-block row (shard-low of quadrant)
COLS_CAP = 128
GMAX = 8

_CACHE = {}


def _plan(edge_index):
    key = hash(edge_index.tobytes())
    if key in _CACHE:
        return _CACHE[key]
    src = np.asarray(edge_index[0], dtype=np.int64)
    dst = np.asarray(edge_index[1], dtype=np.int64)
    deg = (np.bincount(dst, minlength=N) + 1).astype(np.float64)
    dinv = (1.0 / np.sqrt(deg)).astype(np.float32)

    # degree-sorted positions: real nodes at 0..NPC-1 of each core's shard
    pos_of = np.empty(N, dtype=np.int64)
    pes = []
    for c in range(NCORES):
        own = np.arange(c * NPC, (c + 1) * NPC)
        order = np.argsort(-deg[own], kind="stable")
        pos_of[own[order]] = np.arange(NPC)
        pes.append(own[order])

    # per-core edge lists (edges + self loops) and shared per-(block,q) maxima
    per_core = []
    Kbq = np.zeros((NB, 4), dtype=np.int64)
    for c in range(NCORES):
        m = (dst >= c * NPC) & (dst < (c + 1) * NPC)
        s_ = src[m]
        own = np.arange(c * NPC, (c + 1) * NPC)
        ps = np.concatenate([pos_of[dst[m]], pos_of[own]])
        ss = np.concatenate([s_, own])
        qq = ss // (2 * NPC)
        loc = ((ss // NPC) % 2) * SH + pos_of[ss]
        key_e = ps * 4 + qq
        o = np.argsort(key_e, kind="stable")
        sk = key_e[o]
        new = np.r_[True, sk[1:] != sk[:-1]]
        starts = np.flatnonzero(new)
        gid = np.cumsum(new) - 1
        kk = np.empty(len(sk), dtype=np.int64)
        kk[o] = np.arange(len(sk)) - starts[gid]
        cnt = np.bincount(key_e, minlength=NB * P * 4).reshape(NB, P, 4)
        Kbq = np.maximum(Kbq, cnt.max(1))
        per_core.append((ps, qq, kk, loc))

    groups = []
    b = 0
    while b < NB:
        G = 1
        K = Kbq[b].copy()
        while b + G < NB and G < GMAX:
            K2 = np.maximum(K, Kbq[b + G])
            if (G + 1) * int(K2.sum()) > COLS_CAP:
                break
            K = K2
            G += 1
        groups.append((b, G, K.astype(np.int64)))
        b += G

    calls = []
    c16 = 0
    for gi, (bs_, G, K) in enumerate(groups):
        qoff = 0
        for q in range(4):
            Kq = int(K[q])
            if Kq == 0:
                continue
            n = P * G * Kq
            calls.append((gi, q, c16, n, qoff))
            c16 += n // 16
            qoff += G * Kq
    TOTC16 = c16

    # slot-base lookup tables indexed by (group, quadrant)
    gstart = np.empty(NB, dtype=np.int64)
    grp_of = np.empty(NB, dtype=np.int64)
    for gi, (bs_, G, K) in enumerate(groups):
        grp_of[bs_:bs_ + G] = gi
        gstart[bs_:bs_ + G] = bs_
    base_gq = np.full((len(groups), 4), -1, dtype=np.int64)
    K_gq = np.zeros((len(groups), 4), dtype=np.int64)
    for (gi, q, c16s, n, qoff) in calls:
        base_gq[gi, q] = c16s * 16
        K_gq[gi, q] = (n // P) // len(range(groups[gi][0], groups[gi][0] + groups[gi][1]))

    idx16 = []
    for c in range(NCORES):
        ps, qq, kk, loc = per_core[c]
        bb = ps >> 7
        pp = ps & 127
        gi_e = grp_of[bb]
        g_e = bb - gstart[bb]
        Kq_e = K_gq[gi_e, qq]
        slot = base_gq[gi_e, qq] + (g_e * Kq_e + kk) * P + pp
        flat = np.full(TOTC16 * 16, PADLOC, dtype=np.int16)
        flat[slot] = loc.astype(np.int16)
        idx16.append(np.ascontiguousarray(flat.reshape(-1, 16).T))

    dinvbs = []
    x16s_pe = []
    for c in range(NCORES):
        dv = np.zeros(NB * P, dtype=np.float32)
        dv[:NPC] = dinv[pes[c]]
        dinvbs.append(np.ascontiguousarray(dv.reshape(NB, P).T))

    plan = dict(dinv=dinv, pes=pes, groups=groups, calls=calls,
                idx16=idx16, dinvbs=dinvbs, TOTC16=TOTC16)
    _CACHE[key] = plan
    return plan


def _build(plan):
    """One NEFF: L0 table build, then 3x (AllGather, gather, reduce,
    epilogue) with layer-2/3 table builds via PE transpose + matmul."""
    import concourse.bacc as bacc
    import concourse.mybir as mybir
    import concourse.tile as tile
    from concourse.masks import make_identity

    groups, calls, TOTC16 = plan["groups"], plan["calls"], plan["TOTC16"]
    f32 = mybir.dt.float32
    f16 = mybir.dt.float16
    i16 = mybir.dt.int16
    nc = bacc.Bacc("TRN2", target_bir_lowering=False, num_swdge_queues=2)
    xt_in = nc.dram_tensor("xt16", [D, SH], f16, kind="ExternalInput")
    idx_in = nc.dram_tensor("idx16", [16, TOTC16], i16, kind="ExternalInput")
    dinv_in = nc.dram_tensor("dinvb", [P, NB], f32, kind="ExternalInput")
    bias_in = nc.dram_tensor("bias3", [P, 3 * D], f32, kind="ExternalInput")
    w_in = nc.dram_tensor("w16", [D, 3 * D], f16, kind="ExternalInput")
    h_out = nc.dram_tensor("h16", [NB * P, D], f16, kind="ExternalOutput")

    with tile.TileContext(nc) as tc:
        with (
            tc.tile_pool(name="cst", bufs=1) as cst,
            tc.tile_pool(name="wk", bufs=2) as wk,
            tc.tile_pool(name="ep", bufs=2) as ep,
            tc.tile_pool(name="st", bufs=2) as st,
            tc.tile_pool(name="psT", bufs=2, space="PSUM") as psT,
            tc.tile_pool(name="psM", bufs=2, space="PSUM") as psM,
            tc.tile_pool(name="dram", bufs=1, space="DRAM") as dram,
        ):
            tloc = [dram.tile([SH, D], f32, name=f"tloc{i}") for i in range(3)]
            table = [dram.tile([TBL, D], f32, addr_space="Shared",
                               name=f"table{i}") for i in range(3)]

            idx_sb = cst.tile([P, TOTC16], i16)
            for k in range(8):
                nc.sync.dma_start(out=idx_sb[16 * k:16 * (k + 1), :], in_=idx_in[:])
            dinvb = cst.tile([P, NB], f32)
            nc.sync.dma_start(out=dinvb[:], in_=dinv_in[:])
            bias3 = cst.tile([P, 3 * D], f32)
            nc.sync.dma_start(out=bias3[:], in_=bias_in[:])
            w16 = cst.tile([D, 3 * D], f16)
            nc.sync.dma_start(out=w16[:], in_=w_in[:])
            xt16 = cst.tile([D, SH], f16)
            nc.sync.dma_start(out=xt16[:], in_=xt_in[:])
            ident = cst.tile([P, P], f32)
            make_identity(nc, ident[:])
            zb = cst.tile([P, D], f32)
            nc.vector.memset(zb[:], 0.0)
            nc.sync.dma_start(out=tloc[1][NB * P:SH, :], in_=zb[:])
            nc.sync.dma_start(out=tloc[2][NB * P:SH, :], in_=zb[:])

            # L0: tloc[0] rows = (dinv*x) @ W1  (xt16 pre-scaled/transposed)
            for g0 in range(0, NB + 1, 8):
                gn = min(8, NB + 1 - g0)
                pst = psM.tile([P, 8 * D], f32, tag="ps0")
                for j in range(gn):
                    blk = g0 + j
                    nc.tensor.matmul(
                        out=pst[:, j * D:(j + 1) * D],
                        lhsT=xt16[:, blk * P:(blk + 1) * P],
                        rhs=w16[:, 0:D], start=True, stop=True)
                stg = st.tile([P, 8 * D], f32, tag="stg0")
                nc.vector.tensor_copy(out=stg[:, :gn * D], in_=pst[:, :gn * D])
                nc.sync.dma_start(
                    out=tloc[0][g0 * P:(g0 + gn) * P, :]
                        .rearrange("(g p) d -> p g d", p=P),
                    in_=stg[:, :gn * D])

            for lyr in range(3):
                last = lyr == 2
                nc.gpsimd.collective_compute(
                    "AllGather",
                    mybir.AluOpType.bypass,
                    replica_groups=[list(range(NCORES))],
                    ins=[tloc[lyr].opt()],
                    outs=[table[lyr].opt()],
                )
                tbl = table[lyr]
                for gi, (bstart, G, K) in enumerate(groups):
                    COLS = G * int(K.sum())
                    gbuf = wk.tile([P, COLS, D], f32, tag="gbuf")
                    for (gi2, q, c16s, n, qoff) in calls:
                        if gi2 != gi:
                            continue
                        nc.gpsimd.dma_gather(
                            out_ap=gbuf[:, qoff:qoff + n // P, :],
                            in_ap=tbl[q * QROWS:(q + 1) * QROWS, :],
                            idxs_ap=idx_sb[:, c16s:c16s + n // 16],
                            num_idxs=n, num_idxs_reg=n, elem_size=D,
                            single_packet=False, queue_num=q % 2)
                    acc = ep.tile([P, GMAX, D], f32, tag="acc")
                    tmp = ep.tile([P, GMAX, D], f32, tag="tmp")
                    first = True
                    for (gi2, q, c16s, n, qoff) in calls:
                        if gi2 != gi:
                            continue
                        Kq = (n // P) // G
                        red_in = gbuf[:, qoff:qoff + G * Kq, :] \
                            .rearrange("p (g k) d -> p g d k", g=G)
                        nc.vector.tensor_reduce(
                            out=(acc if first else tmp)[:, :G, :], in_=red_in,
                            axis=mybir.AxisListType.X, op=mybir.AluOpType.add)
                        if not first:
                            nc.vector.tensor_tensor(
                                out=acc[:, :G, :], in0=acc[:, :G, :],
                                in1=tmp[:, :G, :], op=mybir.AluOpType.add)
                        first = False
                    dvb = dinvb[:, bstart:bstart + G].to_broadcast([P, G, D])
                    bias = bias3[:, lyr * D:(lyr + 1) * D] \
                        .rearrange("p (g d) -> p g d", g=1).to_broadcast([P, G, D])
                    t1 = ep.tile([P, GMAX, D], f32, tag="t1")
                    nc.vector.tensor_tensor(out=t1[:, :G, :], in0=acc[:, :G, :],
                                            in1=dvb, op=mybir.AluOpType.mult)
                    t2 = ep.tile([P, GMAX, D], f32, tag="t2")
                    nc.vector.tensor_tensor(out=t2[:, :G, :], in0=t1[:, :G, :],
                                            in1=bias, op=mybir.AluOpType.add)
                    h = ep.tile([P, GMAX, D], f32, tag="h")
                    nc.scalar.activation(out=h[:, :G, :], in_=t2[:, :G, :],
                                         func=mybir.ActivationFunctionType.Relu)
                    if last:
                        h16s = ep.tile([P, GMAX, D], f16, tag="h16s")
                        nc.vector.tensor_copy(out=h16s[:, :G, :], in_=h[:, :G, :])
                        nc.sync.dma_start(
                            out=h_out[bstart * P:(bstart + G) * P, :]
                                .rearrange("(g p) d -> p g d", p=P),
                            in_=h16s[:, :G, :])
                    else:
                        hh = ep.tile([P, GMAX, D], f32, tag="hh")
                        nc.vector.tensor_tensor(out=hh[:, :G, :], in0=h[:, :G, :],
                                                in1=dvb, op=mybir.AluOpType.mult)
                        agst = ep.tile([P, GMAX, D], f32, tag="agst")
                        for bqi in range(G):
                            pt = psT.tile([D, P], f32, tag="pt")
                            nc.tensor.transpose(out=pt[:], in_=hh[:, bqi, :],
                                                identity=ident[:])
                            ht = ep.tile([D, P], f16, tag="ht")
                            nc.scalar.copy(out=ht[:], in_=pt[:])
                            pm = psM.tile([P, D], f32, tag="pm")
                            nc.tensor.matmul(
                                out=pm[:], lhsT=ht[:],
                                rhs=w16[:, (lyr + 1) * D:(lyr + 2) * D],
                                start=True, stop=True)
                            nc.vector.tensor_copy(out=agst[:, bqi, :], in_=pm[:])
                        nc.sync.dma_start(
                            out=tloc[lyr + 1][bstart * P:(bstart + G) * P, :]
                                .rearrange("(g p) d -> p g d", p=P),
                            in_=agst[:, :G, :])
    nc.compile()
    return nc


def kernel(x, W1, b1, W2, b2, W3, b3, edge_index):
    import time as _t
    from concourse.bass_utils import run_bass_kernel_spmd as _rb

    x = np.asarray(x, dtype=np.float32)
    Ws = [np.asarray(w, dtype=np.float32) for w in (W1, W2, W3)]
    bs = [np.asarray(b, dtype=np.float32) for b in (b1, b2, b3)]
    plan = _plan(np.asarray(edge_index))
    dinv, pes, dinvbs = plan["dinv"], plan["pes"], plan["dinvbs"]
    cores = list(range(NCORES))

    if "nc" not in plan:
        plan["nc"] = _build(plan)
    nc = plan["nc"]

    bias3 = np.ascontiguousarray(
        np.tile(np.concatenate(bs)[None, :], (P, 1)).astype(np.float32))
    w16 = np.ascontiguousarray(
        np.concatenate(Ws, axis=1).astype(np.float16))
    in_maps = []
    for c in cores:
        pe = pes[c]
        xt = np.zeros((D, SH), dtype=np.float16)
        xt[:, :NPC] = (x[pe] * dinv[pe][:, None]).T.astype(np.float16)
        in_maps.append(dict(xt16=xt, idx16=plan["idx16"][c],
                            dinvb=dinvbs[c], bias3=bias3, w16=w16))

    t0 = _t.time()
    r = _rb(nc, in_maps, core_ids=cores)
    wall_ns = (_t.time() - t0) * 1e9
    t_ns = r.exec_time_ns if r.exec_time_ns is not None else int(wall_ns)

    out = np.empty((N, D), dtype=np.float32)
    for c in cores:
        out[pes[c]] = r.results[c]["h16"][:NPC].astype(np.float32)
    print(f"HW exec time: {t_ns} ns")
    return out
